# revision 20
# baseline (speedup 1.0000x reference)
"""Trainium2 Bass kernel for Bottleneck+DynamicConv (B=16,C=256,H=W=64,E=4).

Data-parallel over batch: 8 NeuronCores x 2 samples each. Each 3x3 conv
uses 1D (vertical) Winograd F(2,3): with G' = [[1,0,0],[1,1,1],[1,-1,1],
[0,0,1]] (integer entries; the usual 1/2 factors are moved to the output
side via the Winograd diagonal-scaling identity), two output rows need 4
comp matmuls per kx-tap instead of 6 taps -> 2/3 the PE work of direct
conv. The vertical input transform is 4 cheap DVE ops per row-block
(contiguous fp16, 4x DVE mode); the output combine is
  Y0 = M0 + (M1+M2)/2,   Y1 = (M1-M2)/2 - M3
done by 4 DVE ops per block straight out of PSUM, then the scalar engine
applies BN bias + SiLU during the write. Per-sample routing pools are
accumulated for free via the activation accum_out port. Compute dtype
fp16 (same PE rate as bf16, 8x finer rounding), accumulation fp32.
"""

from contextlib import ExitStack

import numpy as np

import concourse.bacc as bacc
import concourse.bass as bass
import concourse.mybir as mybir
from concourse import tile
from concourse.bass_utils import run_bass_kernel_spmd

B, C, H, W, E = 16, 256, 64, 64, 4
KH = KW = 3
EPS = 1e-5
NCORES = 8
S = B // NCORES           # samples per core = 2
CT = C // 128             # channel tiles = 2
PD = W + 2                # padded width/height = 66
PF = PD * PD              # padded flat pixels per channel tile = 4356
NI = 4                    # winograd comps (vertical)
TRB = 8                   # tile-rows per block (16 output rows)
NBL = 32 // TRB           # row blocks per image = 4
NN = TRB * W              # matmul free dim = 512
HWF = H * W               # 4096
NWB = CT * NI * KW * CT   # 48 weight blocks of [128,128]
BLKF = NWB * 128          # 6144 weight columns
TCW = TRB * PD            # T comp row-block width = 528
F16 = mybir.dt.float16
F32 = mybir.dt.float32
NPF16 = np.float16
ADD = mybir.AluOpType.add
SUB = mybir.AluOpType.subtract
MUL = mybir.AluOpType.mult
# G' rows applied over ky (integer variant; 1/2 moved to output combine)
GP = np.array([[1, 0, 0], [1, 1, 1], [1, -1, 1], [0, 0, 1]], np.float32)

TRACE = False
LAST_EXEC_NS = None
# swappable for simulator testing (CoreSim has no Silu); HW uses native Silu
ACT_FUNC = mybir.ActivationFunctionType.Silu

_prog_cache = {}


def _wblk(o, i, dx, ci):
    # column block index in the packed weight layout [128, 48*128]
    return ((o * NI + i) * KW + dx) * CT + ci


def _emit_conv(nc, pools, wcol, pview, epilogue):
    """One winograd conv layer for one sample.
    wcol(o,i,dx,ci) -> [128,128] lhsT AP; pview = [ci][128, 66, 66] padded
    fp16 input APs; epilogue(o, blk, comps[4])."""
    t_pool, ps_pool = pools
    for blk in range(NBL):
        r0 = 2 * blk * TRB  # first padded image row of the block
        tt = t_pool.tile([128, NI * CT * TCW], F16, tag="T")

        def tcomp(i, ci, tt=tt):
            c0 = (i * CT + ci) * TCW
            return tt[:, c0:c0 + TCW].rearrange("p (r w) -> p r w", w=PD)

        for ci in range(CT):
            rows = [pview[ci][:, r0 + a:r0 + a + 2 * TRB - 1:2, :]
                    for a in range(4)]
            nc.vector.tensor_tensor(tcomp(0, ci), rows[0], rows[2], SUB)
            nc.gpsimd.tensor_tensor(tcomp(1, ci), rows[1], rows[2], ADD)
            nc.vector.tensor_tensor(tcomp(2, ci), rows[2], rows[1], SUB)
            nc.vector.tensor_tensor(tcomp(3, ci), rows[1], rows[3], SUB)
        for o in range(CT):
            comps = []
            for i in range(NI):
                ps = ps_pool.tile([128, NN], F32, tag="ps")
                comps.append(ps)
                idx = 0
                for dx in range(KW):
                    for ci in range(CT):
                        nc.tensor.matmul(
                            ps[:], wcol(o, i, dx, ci),
                            tcomp(i, ci)[:, :, dx:dx + W],
                            start=(idx == 0), stop=(idx == 5))
                        idx += 1
            epilogue(o, blk, comps)


def _build_program():
    nc = bacc.Bacc(
        "TRN2", target_bir_lowering=False, debug=False,
        enable_asserts=False, num_devices=NCORES)

    xpad_d = nc.dram_tensor("xpad", [S, CT, 128, PF], F16, kind="ExternalInput")
    w1_d = nc.dram_tensor("w1sb", [128, BLKF], F16, kind="ExternalInput")
    bank_d = nc.dram_tensor("bank", [128, E * BLKF], F16, kind="ExternalInput")
    wr_d = nc.dram_tensor("wrt", [128, CT * E], F32, kind="ExternalInput")
    br_d = nc.dram_tensor("brb", [128, E], F32, kind="ExternalInput")
    b1_d = nc.dram_tensor("b1sb", [128, CT], F32, kind="ExternalInput")
    b2_d = nc.dram_tensor("b2sb", [128, CT], F32, kind="ExternalInput")
    out_d = nc.dram_tensor("out", [S, CT, 128, HWF], F32, kind="ExternalOutput")

    with tile.TileContext(nc) as tc, ExitStack() as ctx:
        const = ctx.enter_context(tc.tile_pool(name="const", bufs=1))
        xp_pool = ctx.enter_context(tc.tile_pool(name="xp", bufs=2))
        yp_pool = ctx.enter_context(tc.tile_pool(name="yp", bufs=2))
        t_pool = ctx.enter_context(tc.tile_pool(name="tp", bufs=2))
        kern_pool = ctx.enter_context(tc.tile_pool(name="kern", bufs=2))
        ep_pool = ctx.enter_context(tc.tile_pool(name="ep", bufs=2))
        outc_pool = ctx.enter_context(tc.tile_pool(name="outc", bufs=2))
        small = ctx.enter_context(tc.tile_pool(name="small", bufs=4))
        ps_pool = ctx.enter_context(tc.tile_pool(name="ps", bufs=7, space="PSUM"))
        psr_pool = ctx.enter_context(tc.tile_pool(name="psr", bufs=1, space="PSUM"))
        pools = (t_pool, ps_pool)

        # startup-critical DMA order: the very first matmuls need only the
        # o=0 weight blocks and the leading image rows of sample 0 — load
        # those first so the PE starts early
        w1_t = const.tile([128, BLKF], F16)
        nc.scalar.dma_start(w1_t[:, 0:BLKF // 2], w1_d.ap()[:, 0:BLKF // 2])
        b1_t = const.tile([128, CT], F32)
        nc.sync.dma_start(b1_t[:], b1_d.ap())
        wr_t = const.tile([128, CT * E], F32)
        br_t = const.tile([128, E], F32)
        b2_t = const.tile([128, CT], F32)
        ones_t = const.tile([128, 128], F32)
        nc.vector.memset(ones_t[:], 1.0)
        # the big expert bank is only needed after conv1(s0): allocate now,
        # DMA later so it doesn't delay the startup-critical loads above
        bank_t = const.tile([128, E * BLKF], F16)

        # s0 image quarters land in consumption order (row-blocks need rows
        # <18, <34, <50, <66 respectively)
        QS = [0, 18 * PD, 34 * PD, 50 * PD, PF]
        xpts, ypts, kerns = [], [], []
        for s in range(S):
            xpt = xp_pool.tile([128, CT * PF], F16, tag="xp")
            xpts.append(xpt)
            pieces = (list(zip(QS[:-1], QS[1:])) if s == 0 else [(0, PF)])
            for qi, (lo, hi) in enumerate(pieces):
                # split image quarters across both HWDGE rings (SP + ACT)
                for ci in range(CT):
                    eng = nc.sync if ci == 0 else nc.scalar
                    eng.dma_start(
                        xpt[:, ci * PF + lo:ci * PF + hi],
                        xpad_d.ap()[s, ci][:, lo:hi])
                if s == 0 and qi == 1:
                    nc.scalar.dma_start(
                        w1_t[:, BLKF // 2:], w1_d.ap()[:, BLKF // 2:])
            if s == 0:
                nc.sync.dma_start(wr_t[:], wr_d.ap())
                nc.sync.dma_start(br_t[:], br_d.ap())
                nc.sync.dma_start(b2_t[:], b2_d.ap())
            xviews = [xpt[:, ci * PF:(ci + 1) * PF].rearrange(
                "p (h w) -> p h w", h=PD) for ci in range(CT)]

            ypt = yp_pool.tile([128, CT * PF], F16, tag="yp")
            ypts.append(ypt)
            yviews = [ypt[:, ci * PF:(ci + 1) * PF].rearrange(
                "p (h w) -> p h w", h=PD) for ci in range(CT)]
            # zero the pad ring up front; epilogues only write the interior
            for ci in range(CT):
                yv = yviews[ci]
                nc.vector.memset(yv[:, 0:1, :], 0.0)
                nc.vector.memset(yv[:, PD - 1:PD, :], 0.0)
                nc.vector.memset(yv[:, :, 0:1], 0.0)
                nc.vector.memset(yv[:, :, PD - 1:PD], 0.0)

            # per-epilogue partial row sums of y, accumulated by the
            # activations for free (accum_out) — feeds the routing pool
            pp_t = small.tile([128, CT * NBL * 2], F32, tag="pp")

            def epi1(o, blk, comps, yviews=yviews, pp_t=pp_t):
                # evacuate the 4 comps PSUM->SBUF on otherwise-idle DMA
                # rings, then combine all-SBUF:
                #   2*Y0 = 2*m0 + (m1+m2) ; -2*Y1 = 2*m3 - (m1-m2)
                # with the +-0.5 folded into the activation scale.
                m0, m1, m2, m3 = comps
                t1 = ep_pool.tile([128, NN], F32, tag="a", name="t1")
                t2 = ep_pool.tile([128, NN], F32, tag="b", name="t2")
                u = ep_pool.tile([128, NN], F32, tag="u")
                v = ep_pool.tile([128, NN], F32, tag="v")
                y0 = ep_pool.tile([128, NN], F32, tag="a", name="y0")
                y1n = ep_pool.tile([128, NN], F32, tag="b", name="y1n")
                cp = mybir.ActivationFunctionType.Copy
                nc.scalar.activation(t1[:], m1[:], cp, scale=0.5)
                nc.scalar.activation(t2[:], m2[:], cp, scale=0.5)
                nc.vector.tensor_tensor(u[:], t1[:], t2[:], ADD)
                nc.vector.tensor_tensor(v[:], t1[:], t2[:], SUB)
                nc.vector.tensor_tensor(y0[:], m0[:], u[:], ADD)
                nc.vector.tensor_tensor(y1n[:], m3[:], v[:], SUB)
                r0 = 2 * blk * TRB + 1  # first unpadded out row, padded coord
                for a, yf, sc in ((0, y0, 1.0), (1, y1n, -1.0)):
                    nc.scalar.activation(
                        yviews[o][:, r0 + a:r0 + a + 2 * TRB:2, 1:1 + W],
                        yf[:].rearrange("p (r w) -> p r w", r=TRB),
                        ACT_FUNC, bias=b1_t[:, o:o + 1], scale=sc,
                        accum_out=pp_t[:, (o * NBL + blk) * 2 + a:
                                       (o * NBL + blk) * 2 + a + 1])

            _emit_conv(
                nc, pools,
                lambda o, i, dx, ci: w1_t[:, _wblk(o, i, dx, ci) * 128:
                                          (_wblk(o, i, dx, ci) + 1) * 128],
                xviews, epi1)

            if s == 0:
                nc.sync.dma_start(bank_t[:], bank_d.ap())

            # routing: pooled mean -> sigmoid(pooled @ wrT + br); all 128
            # partitions carry identical copies (broadcast via ones-matmul)
            psr = psr_pool.tile([128, E], F32, tag="psr")
            for ci in range(CT):
                pooled = small.tile([128, 1], F32, tag="pooled")
                nc.vector.tensor_reduce(
                    pooled[:], pp_t[:, ci * NBL * 2:(ci + 1) * NBL * 2],
                    axis=mybir.AxisListType.X, op=ADD)
                pbc = small.tile([128, 128], F32, tag="pbc")
                nc.vector.tensor_scalar_mul(pbc[:], ones_t[:], pooled[:, 0:1])
                nc.tensor.matmul(
                    psr[:], pbc[:], wr_t[:, ci * E:(ci + 1) * E],
                    start=(ci == 0), stop=(ci == CT - 1))
            logits = small.tile([128, E], F32, tag="logits")
            nc.vector.tensor_add(logits[:], psr[:], br_t[:])
            # sigmoid(x) = 0.5*tanh(x/2) + 0.5 — tanh shares the silu act
            # table set, avoiding two ~1.3us act-table reloads per sample
            th = small.tile([128, E], F32, tag="th")
            nc.scalar.activation(
                th[:], logits[:], mybir.ActivationFunctionType.Tanh,
                scale=0.5)
            r_t = small.tile([128, E], F32, tag="r")
            nc.vector.tensor_scalar(r_t[:], th[:], 0.5, 0.5, MUL, ADD)

            # expert-weighted kernel bank mix: kern = sum_e r_e * bank_e
            kt = kern_pool.tile([128, BLKF], F16, tag="kern")
            kerns.append(kt)
            nc.vector.tensor_scalar_mul(kt[:], bank_t[:, 0:BLKF], r_t[:, 0:1])
            for e in range(1, E):
                nc.vector.scalar_tensor_tensor(
                    kt[:], bank_t[:, e * BLKF:(e + 1) * BLKF], r_t[:, e:e + 1],
                    kt[:], MUL, ADD)

        for s in range(S):
            ypt, kt = ypts[s], kerns[s]
            yviews = [ypt[:, ci * PF:(ci + 1) * PF].rearrange(
                "p (h w) -> p h w", h=PD) for ci in range(CT)]
            xpt = xpts[s]
            xviews2 = [xpt[:, ci * PF:(ci + 1) * PF].rearrange(
                "p (h w) -> p h w", h=PD) for ci in range(CT)]

            def epi2(o, blk, comps, s=s, xviews2=xviews2):
                m0, m1, m2, m3 = comps
                t1 = ep_pool.tile([128, NN], F32, tag="a", name="t1")
                t2 = ep_pool.tile([128, NN], F32, tag="b", name="t2")
                u = ep_pool.tile([128, NN], F32, tag="u")
                v = ep_pool.tile([128, NN], F32, tag="v")
                y0 = ep_pool.tile([128, NN], F32, tag="a", name="y0")
                y1n = ep_pool.tile([128, NN], F32, tag="b", name="y1n")
                cp = mybir.ActivationFunctionType.Copy
                nc.scalar.activation(t1[:], m1[:], cp, scale=0.5)
                nc.scalar.activation(t2[:], m2[:], cp, scale=0.5)
                nc.vector.tensor_tensor(u[:], t1[:], t2[:], ADD)
                nc.vector.tensor_tensor(v[:], t1[:], t2[:], SUB)
                nc.vector.tensor_tensor(y0[:], m0[:], u[:], ADD)
                nc.vector.tensor_tensor(y1n[:], m3[:], v[:], SUB)
                # silu(bn2) into an interleaved row chunk, add the residual,
                # stream the chunk to DRAM
                oc = outc_pool.tile([128, 2 * NN], F32, tag="outc")
                ocv = oc[:].rearrange("p (r aw) -> p r aw", aw=2 * W)
                for a, yf, sc in ((0, y0, 1.0), (1, y1n, -1.0)):
                    nc.scalar.activation(
                        ocv[:, :, a * W:(a + 1) * W],
                        yf[:].rearrange("p (r w) -> p r w", r=TRB),
                        ACT_FUNC, bias=b2_t[:, o:o + 1], scale=sc)
                r0 = 2 * blk * TRB + 1
                nc.gpsimd.tensor_tensor(
                    oc[:].rearrange("p (r w) -> p r w", w=W),
                    oc[:].rearrange("p (r w) -> p r w", w=W),
                    xviews2[o][:, r0:r0 + 2 * TRB, 1:1 + W], ADD)
                nc.sync.dma_start(
                    out_d.ap()[s, o][:, 2 * blk * NN:2 * (blk + 1) * NN],
                    oc[:])

            _emit_conv(
                nc, pools,
                lambda o, i, dx, ci: kt[:, _wblk(o, i, dx, ci) * 128:
                                        (_wblk(o, i, dx, ci) + 1) * 128],
                yviews, epi2)

    nc.compile()
    return nc


def _get_program():
    if "nc" not in _prog_cache:
        _prog_cache["nc"] = _build_program()
    return _prog_cache["nc"]


def _pack_winograd(wf):
    """[cout, cin, ky, kx] -> lhsT layout [128, (o, i, dx, ci, cout128)]."""
    u = np.einsum('iy,ocyx->ocix', GP, wf)
    return np.ascontiguousarray(
        u.reshape(CT, 128, CT, 128, NI, KW)
        .transpose(3, 0, 4, 5, 2, 1).reshape(128, BLKF)).astype(NPF16)


def kernel(x, w1, bn1_g, bn1_b, bn1_m, bn1_v, wr, br, w_e,
           bn2_g, bn2_b, bn2_m, bn2_v):
    global LAST_EXEC_NS
    f32 = np.float32
    x = np.ascontiguousarray(np.asarray(x, f32))
    w1 = np.asarray(w1, f32)
    wr = np.asarray(wr, f32)
    br = np.asarray(br, f32)
    w_e = np.asarray(w_e, f32)

    s1 = np.asarray(bn1_g, f32) / np.sqrt(np.asarray(bn1_v, f32) + EPS)
    b1 = np.asarray(bn1_b, f32) - np.asarray(bn1_m, f32) * s1
    s2 = np.asarray(bn2_g, f32) / np.sqrt(np.asarray(bn2_v, f32) + EPS)
    b2 = np.asarray(bn2_b, f32) - np.asarray(bn2_m, f32) * s2

    # conv1 weights (BN1 scale folded) -> vertical-winograd lhsT blocks
    w1sb = _pack_winograd(w1 * s1[:, None, None, None])
    # expert bank likewise (BN2 scale folded); transform commutes with the
    # routing mix, so each expert is pre-transformed on the host
    wef = w_e.reshape(E, C, C, KH, KW) * s2[None, :, None, None, None]
    bank = np.concatenate(
        [_pack_winograd(wef[e]) for e in range(E)], axis=1)
    bank = np.ascontiguousarray(bank)

    # routing weights with the 1/(H*W) mean folded in: [p, (ci, e)]
    wrt = np.ascontiguousarray(
        (wr / HWF).reshape(E, CT, 128).transpose(2, 1, 0).reshape(128, CT * E))
    brb = np.ascontiguousarray(np.broadcast_to(br, (128, E)))
    b1sb = np.ascontiguousarray(b1.reshape(CT, 128).T)
    b2sb = np.ascontiguousarray(b2.reshape(CT, 128).T)

    # padded fp16 x for the conv matmuls (also reused as the residual)
    pad = np.zeros((B, CT, 128, PD, PD), f32)
    pad[:, :, :, 1:H + 1, 1:W + 1] = x.reshape(B, CT, 128, H, W)
    xpad = np.ascontiguousarray(pad.reshape(B, CT, 128, PF).astype(NPF16))

    nc = _get_program()
    in_maps = []
    for c in range(NCORES):
        sl = slice(S * c, S * (c + 1))
        in_maps.append({
            "xpad": np.ascontiguousarray(xpad[sl]),
            "w1sb": w1sb, "bank": bank, "wrt": wrt, "brb": brb,
            "b1sb": b1sb, "b2sb": b2sb,
        })

    res = run_bass_kernel_spmd(
        nc, in_maps, core_ids=list(range(NCORES)), trace=TRACE)
    LAST_EXEC_NS = res.exec_time_ns

    out = np.empty((B, C, H, W), f32)
    for c in range(NCORES):
        out[S * c:S * (c + 1)] = res.results[c]["out"].reshape(S, C, H, W)
    return out


if __name__ == "__main__":
    rng = np.random.default_rng(0)
    ins = {
        "x": rng.standard_normal((B, C, H, W), f32 := np.float32),
        "w1": rng.standard_normal((C, C, KH, KW), f32) * 0.05,
        "bn1_g": np.ones(C, f32), "bn1_b": np.zeros(C, f32),
        "bn1_m": rng.standard_normal(C, f32) * 0.05,
        "bn1_v": np.abs(rng.standard_normal(C, f32) * 0.05) + 1.0,
        "wr": rng.standard_normal((E, C), f32) * 0.05,
        "br": np.zeros(E, f32),
        "w_e": rng.standard_normal((E, C * C * KH * KW), f32) * 0.05,
        "bn2_g": np.ones(C, f32), "bn2_b": np.zeros(C, f32),
        "bn2_m": rng.standard_normal(C, f32) * 0.05,
        "bn2_v": np.abs(rng.standard_normal(C, f32) * 0.05) + 1.0,
    }
    o = kernel(**ins)
    print(o.shape, o.dtype)


# revision 23
# speedup vs baseline: 1.0411x; 1.0411x over previous
"""Trainium2 Bass kernel for Bottleneck+DynamicConv (B=16,C=256,H=W=64,E=4).

Data-parallel over batch: 8 NeuronCores x 2 samples each. Each 3x3 conv
uses 1D (vertical) Winograd F(2,3): with G' = [[1,0,0],[1,1,1],[1,-1,1],
[0,0,1]] (integer entries; the usual 1/2 factors are moved to the output
side via the Winograd diagonal-scaling identity), two output rows need 4
comp matmuls per kx-tap instead of 6 taps -> 2/3 the PE work of direct
conv. The vertical input transform is 4 cheap DVE ops per row-block
(contiguous fp16, 4x DVE mode); the output combine is
  Y0 = M0 + (M1+M2)/2,   Y1 = (M1-M2)/2 - M3
done by 4 DVE ops per block straight out of PSUM, then the scalar engine
applies BN bias + SiLU during the write. Per-sample routing pools are
accumulated for free via the activation accum_out port. Compute dtype
fp16 (same PE rate as bf16, 8x finer rounding), accumulation fp32.
"""

from contextlib import ExitStack

import numpy as np

import concourse.bacc as bacc
import concourse.bass as bass
import concourse.mybir as mybir
from concourse import tile
from concourse.bass_utils import run_bass_kernel_spmd

B, C, H, W, E = 16, 256, 64, 64, 4
KH = KW = 3
EPS = 1e-5
NCORES = 8
S = B // NCORES           # samples per core = 2
CT = C // 128             # channel tiles = 2
PD = W + 2                # padded width/height = 66
PF = PD * PD              # padded flat pixels per channel tile = 4356
NI = 4                    # winograd comps (vertical)
TRB = 8                   # tile-rows per block (16 output rows)
NBL = 32 // TRB           # row blocks per image = 4
NN = TRB * W              # matmul free dim = 512
HWF = H * W               # 4096
NWB = CT * NI * KW * CT   # 48 weight blocks of [128,128]
BLKF = NWB * 128          # 6144 weight columns
TCW = TRB * PD            # T comp row-block width = 528
F16 = mybir.dt.float16
F32 = mybir.dt.float32
NPF16 = np.float16
ADD = mybir.AluOpType.add
SUB = mybir.AluOpType.subtract
MUL = mybir.AluOpType.mult
# G' rows applied over ky (integer variant; 1/2 moved to output combine)
GP = np.array([[1, 0, 0], [1, 1, 1], [1, -1, 1], [0, 0, 1]], np.float32)

TRACE = False
LAST_EXEC_NS = None
# swappable for simulator testing (CoreSim has no Silu); HW uses native Silu
ACT_FUNC = mybir.ActivationFunctionType.Silu

_prog_cache = {}


def _wblk(o, i, dx, ci):
    # column block index in the packed weight layout [128, 48*128]
    return ((o * NI + i) * KW + dx) * CT + ci


def _emit_conv(nc, pools, wcol, pview, epilogue, bg_work=None):
    """One winograd conv layer for one sample.
    wcol(o,i,dx,ci) -> [128,128] lhsT AP; pview = [ci][128, 66, 66] padded
    fp16 input APs; epilogue(o, blk, comps[4]). bg_work: deferred closures
    (e.g. kernel-mix chunks) drained a few at a time between groups so big
    DVE ops never block the in-order queue when the conv needs it."""
    t_pool, ps_pool = pools
    bg_work = bg_work if bg_work is not None else []
    for blk in range(NBL):
        r0 = 2 * blk * TRB  # first padded image row of the block
        tt = t_pool.tile([128, NI * CT * TCW], F16, tag="T")

        def tcomp(i, ci, tt=tt):
            c0 = (i * CT + ci) * TCW
            return tt[:, c0:c0 + TCW].rearrange("p (r w) -> p r w", w=PD)

        for ci in range(CT):
            rows = [pview[ci][:, r0 + a:r0 + a + 2 * TRB - 1:2, :]
                    for a in range(4)]
            nc.vector.tensor_tensor(tcomp(0, ci), rows[0], rows[2], SUB)
            nc.vector.tensor_tensor(tcomp(1, ci), rows[1], rows[2], ADD)
            nc.vector.tensor_tensor(tcomp(2, ci), rows[2], rows[1], SUB)
            nc.vector.tensor_tensor(tcomp(3, ci), rows[1], rows[3], SUB)
        for o in range(CT):
            comps = []
            for i in range(NI):
                ps = ps_pool.tile([128, NN], F32, tag="ps")
                comps.append(ps)
                idx = 0
                for dx in range(KW):
                    for ci in range(CT):
                        nc.tensor.matmul(
                            ps[:], wcol(o, i, dx, ci),
                            tcomp(i, ci)[:, :, dx:dx + W],
                            start=(idx == 0), stop=(idx == 5))
                        idx += 1
            epilogue(o, blk, comps)
            for _ in range(6):
                if bg_work:
                    bg_work.pop(0)()


def _build_program():
    nc = bacc.Bacc(
        "TRN2", target_bir_lowering=False, debug=False,
        enable_asserts=False, num_devices=NCORES)

    xpad_d = nc.dram_tensor("xpad", [S, CT, 128, PF], F16, kind="ExternalInput")
    w1_d = nc.dram_tensor("w1sb", [128, BLKF], F16, kind="ExternalInput")
    bank_d = nc.dram_tensor("bank", [128, E * BLKF], F16, kind="ExternalInput")
    wr_d = nc.dram_tensor("wrt", [128, CT * E], F32, kind="ExternalInput")
    br_d = nc.dram_tensor("brb", [128, E], F32, kind="ExternalInput")
    b1_d = nc.dram_tensor("b1sb", [128, CT], F32, kind="ExternalInput")
    b2_d = nc.dram_tensor("b2sb", [128, CT], F32, kind="ExternalInput")
    out_d = nc.dram_tensor("out", [S, CT, 128, HWF], F32, kind="ExternalOutput")

    with tile.TileContext(nc) as tc, ExitStack() as ctx:
        const = ctx.enter_context(tc.tile_pool(name="const", bufs=1))
        xp_pool = ctx.enter_context(tc.tile_pool(name="xp", bufs=2))
        yp_pool = ctx.enter_context(tc.tile_pool(name="yp", bufs=2))
        t_pool = ctx.enter_context(tc.tile_pool(name="tp", bufs=2))
        kern_pool = ctx.enter_context(tc.tile_pool(name="kern", bufs=2))
        ep_pool = ctx.enter_context(tc.tile_pool(name="ep", bufs=2))
        outc_pool = ctx.enter_context(tc.tile_pool(name="outc", bufs=2))
        small = ctx.enter_context(tc.tile_pool(name="small", bufs=4))
        ps_pool = ctx.enter_context(tc.tile_pool(name="ps", bufs=7, space="PSUM"))
        psr_pool = ctx.enter_context(tc.tile_pool(name="psr", bufs=1, space="PSUM"))
        pools = (t_pool, ps_pool)

        # startup-critical DMA order: the very first matmuls need only the
        # o=0 weight blocks and the leading image rows of sample 0 — load
        # those first so the PE starts early
        w1_t = const.tile([128, BLKF], F16)
        nc.scalar.dma_start(w1_t[:, 0:BLKF // 2], w1_d.ap()[:, 0:BLKF // 2])
        b1_t = const.tile([128, CT], F32)
        nc.sync.dma_start(b1_t[:], b1_d.ap())
        wr_t = const.tile([128, CT * E], F32)
        br_t = const.tile([128, E], F32)
        b2_t = const.tile([128, CT], F32)
        ones_t = const.tile([128, 128], F32)
        nc.vector.memset(ones_t[:], 1.0)
        # the big expert bank is only needed after conv1(s0): allocate now,
        # DMA later so it doesn't delay the startup-critical loads above
        bank_t = const.tile([128, E * BLKF], F16)

        # s0 image quarters land in consumption order (row-blocks need rows
        # <18, <34, <50, <66 respectively)
        QS = [0, 18 * PD, 34 * PD, 50 * PD, PF]
        xpts, ypts, kerns = [], [], []
        mix_lists = []
        for s in range(S):
            xpt = xp_pool.tile([128, CT * PF], F16, tag="xp")
            xpts.append(xpt)
            pieces = (list(zip(QS[:-1], QS[1:])) if s == 0 else [(0, PF)])
            for qi, (lo, hi) in enumerate(pieces):
                # split image quarters across both HWDGE rings (SP + ACT)
                for ci in range(CT):
                    eng = nc.sync if ci == 0 else nc.scalar
                    eng.dma_start(
                        xpt[:, ci * PF + lo:ci * PF + hi],
                        xpad_d.ap()[s, ci][:, lo:hi])
                if s == 0 and qi == 1:
                    nc.scalar.dma_start(
                        w1_t[:, BLKF // 2:], w1_d.ap()[:, BLKF // 2:])
            if s == 0:
                nc.sync.dma_start(wr_t[:], wr_d.ap())
                nc.sync.dma_start(br_t[:], br_d.ap())
                nc.sync.dma_start(b2_t[:], b2_d.ap())
            xviews = [xpt[:, ci * PF:(ci + 1) * PF].rearrange(
                "p (h w) -> p h w", h=PD) for ci in range(CT)]

            ypt = yp_pool.tile([128, CT * PF], F16, tag="yp")
            ypts.append(ypt)
            yviews = [ypt[:, ci * PF:(ci + 1) * PF].rearrange(
                "p (h w) -> p h w", h=PD) for ci in range(CT)]
            # zero the pad ring up front; epilogues only write the interior
            for ci in range(CT):
                yv = yviews[ci]
                nc.vector.memset(yv[:, 0:1, :], 0.0)
                nc.vector.memset(yv[:, PD - 1:PD, :], 0.0)
                nc.vector.memset(yv[:, :, 0:1], 0.0)
                nc.vector.memset(yv[:, :, PD - 1:PD], 0.0)

            # per-epilogue partial row sums of y, accumulated by the
            # activations for free (accum_out) — feeds the routing pool
            pp_t = small.tile([128, CT * NBL * 2], F32, tag="pp")

            def epi1(o, blk, comps, yviews=yviews, pp_t=pp_t):
                # evacuate the 4 comps PSUM->SBUF on otherwise-idle DMA
                # rings, then combine all-SBUF:
                #   2*Y0 = 2*m0 + (m1+m2) ; -2*Y1 = 2*m3 - (m1-m2)
                # with the +-0.5 folded into the activation scale.
                m0, m1, m2, m3 = comps
                t1 = ep_pool.tile([128, NN], F32, tag="a", name="t1")
                t2 = ep_pool.tile([128, NN], F32, tag="b", name="t2")
                u = ep_pool.tile([128, NN], F32, tag="u")
                v = ep_pool.tile([128, NN], F32, tag="v")
                y0 = ep_pool.tile([128, NN], F32, tag="a", name="y0")
                y1n = ep_pool.tile([128, NN], F32, tag="b", name="y1n")
                cp = mybir.ActivationFunctionType.Copy
                nc.scalar.activation(t1[:], m1[:], cp, scale=0.5)
                nc.scalar.activation(t2[:], m2[:], cp, scale=0.5)
                nc.vector.tensor_tensor(u[:], t1[:], t2[:], ADD)
                nc.vector.tensor_tensor(v[:], t1[:], t2[:], SUB)
                nc.vector.tensor_tensor(y0[:], m0[:], u[:], ADD)
                nc.vector.tensor_tensor(y1n[:], m3[:], v[:], SUB)
                r0 = 2 * blk * TRB + 1  # first unpadded out row, padded coord
                for a, yf, sc in ((0, y0, 1.0), (1, y1n, -1.0)):
                    nc.scalar.activation(
                        yviews[o][:, r0 + a:r0 + a + 2 * TRB:2, 1:1 + W],
                        yf[:].rearrange("p (r w) -> p r w", r=TRB),
                        ACT_FUNC, bias=b1_t[:, o:o + 1], scale=sc,
                        accum_out=pp_t[:, (o * NBL + blk) * 2 + a:
                                       (o * NBL + blk) * 2 + a + 1])

            _emit_conv(
                nc, pools,
                lambda o, i, dx, ci: w1_t[:, _wblk(o, i, dx, ci) * 128:
                                          (_wblk(o, i, dx, ci) + 1) * 128],
                xviews, epi1,
                mix_lists[s - 1] if s >= 1 else None)

            if s == 0:
                nc.sync.dma_start(bank_t[:], bank_d.ap())

            # routing: pooled mean -> sigmoid(pooled @ wrT + br); all 128
            # partitions carry identical copies (broadcast via ones-matmul)
            psr = psr_pool.tile([128, E], F32, tag="psr")
            for ci in range(CT):
                pooled = small.tile([128, 1], F32, tag="pooled")
                nc.vector.tensor_reduce(
                    pooled[:], pp_t[:, ci * NBL * 2:(ci + 1) * NBL * 2],
                    axis=mybir.AxisListType.X, op=ADD)
                pbc = small.tile([128, 128], F32, tag="pbc")
                nc.vector.tensor_scalar_mul(pbc[:], ones_t[:], pooled[:, 0:1])
                nc.tensor.matmul(
                    psr[:], pbc[:], wr_t[:, ci * E:(ci + 1) * E],
                    start=(ci == 0), stop=(ci == CT - 1))
            logits = small.tile([128, E], F32, tag="logits")
            nc.vector.tensor_add(logits[:], psr[:], br_t[:])
            # sigmoid(x) = 0.5*tanh(x/2) + 0.5 — tanh shares the silu act
            # table set, avoiding two ~1.3us act-table reloads per sample
            th = small.tile([128, E], F32, tag="th")
            nc.scalar.activation(
                th[:], logits[:], mybir.ActivationFunctionType.Tanh,
                scale=0.5)
            r_t = small.tile([128, E], F32, tag="r")
            nc.vector.tensor_scalar(r_t[:], th[:], 0.5, 0.5, MUL, ADD)

            # expert-weighted kernel bank mix: kern = sum_e r_e * bank_e,
            # emitted as deferred 512-col chunks drained between the next
            # conv's groups — a monolithic mix blocks the in-order DVE
            # queue for ~7us right when the conv needs transform/epi ops
            kt = kern_pool.tile([128, BLKF], F16, tag="kern")
            kerns.append(kt)
            mix_list = []
            mix_lists.append(mix_list)

            def _mix_chunk(lo, hi, e, kt=kt, r_t=r_t):
                def run():
                    if e == 0:
                        nc.vector.tensor_scalar_mul(
                            kt[:, lo:hi], bank_t[:, lo:hi], r_t[:, 0:1])
                    else:
                        nc.vector.scalar_tensor_tensor(
                            kt[:, lo:hi],
                            bank_t[:, e * BLKF + lo:e * BLKF + hi],
                            r_t[:, e:e + 1], kt[:, lo:hi], MUL, ADD)
                return run

            MC = BLKF // 12
            for k in range(12):
                for e in range(E):
                    mix_list.append(_mix_chunk(k * MC, (k + 1) * MC, e))

        for s in range(S):
            ypt, kt = ypts[s], kerns[s]
            yviews = [ypt[:, ci * PF:(ci + 1) * PF].rearrange(
                "p (h w) -> p h w", h=PD) for ci in range(CT)]
            xpt = xpts[s]
            xviews2 = [xpt[:, ci * PF:(ci + 1) * PF].rearrange(
                "p (h w) -> p h w", h=PD) for ci in range(CT)]

            while mix_lists[s]:
                mix_lists[s].pop(0)()

            def epi2(o, blk, comps, s=s, xviews2=xviews2):
                m0, m1, m2, m3 = comps
                t1 = ep_pool.tile([128, NN], F32, tag="a", name="t1")
                t2 = ep_pool.tile([128, NN], F32, tag="b", name="t2")
                u = ep_pool.tile([128, NN], F32, tag="u")
                v = ep_pool.tile([128, NN], F32, tag="v")
                y0 = ep_pool.tile([128, NN], F32, tag="a", name="y0")
                y1n = ep_pool.tile([128, NN], F32, tag="b", name="y1n")
                cp = mybir.ActivationFunctionType.Copy
                nc.scalar.activation(t1[:], m1[:], cp, scale=0.5)
                nc.scalar.activation(t2[:], m2[:], cp, scale=0.5)
                nc.vector.tensor_tensor(u[:], t1[:], t2[:], ADD)
                nc.vector.tensor_tensor(v[:], t1[:], t2[:], SUB)
                nc.vector.tensor_tensor(y0[:], m0[:], u[:], ADD)
                nc.vector.tensor_tensor(y1n[:], m3[:], v[:], SUB)
                # silu(bn2) into an interleaved row chunk, add the residual,
                # stream the chunk to DRAM
                oc = outc_pool.tile([128, 2 * NN], F32, tag="outc")
                ocv = oc[:].rearrange("p (r aw) -> p r aw", aw=2 * W)
                for a, yf, sc in ((0, y0, 1.0), (1, y1n, -1.0)):
                    nc.scalar.activation(
                        ocv[:, :, a * W:(a + 1) * W],
                        yf[:].rearrange("p (r w) -> p r w", r=TRB),
                        ACT_FUNC, bias=b2_t[:, o:o + 1], scale=sc)
                r0 = 2 * blk * TRB + 1
                nc.vector.tensor_add(
                    oc[:].rearrange("p (r w) -> p r w", w=W),
                    oc[:].rearrange("p (r w) -> p r w", w=W),
                    xviews2[o][:, r0:r0 + 2 * TRB, 1:1 + W])
                nc.sync.dma_start(
                    out_d.ap()[s, o][:, 2 * blk * NN:2 * (blk + 1) * NN],
                    oc[:])

            _emit_conv(
                nc, pools,
                lambda o, i, dx, ci: kt[:, _wblk(o, i, dx, ci) * 128:
                                        (_wblk(o, i, dx, ci) + 1) * 128],
                yviews, epi2,
                mix_lists[s + 1] if s + 1 < S else None)

    nc.compile()
    return nc


def _get_program():
    if "nc" not in _prog_cache:
        _prog_cache["nc"] = _build_program()
    return _prog_cache["nc"]


def _pack_winograd(wf):
    """[cout, cin, ky, kx] -> lhsT layout [128, (o, i, dx, ci, cout128)]."""
    u = np.einsum('iy,ocyx->ocix', GP, wf)
    return np.ascontiguousarray(
        u.reshape(CT, 128, CT, 128, NI, KW)
        .transpose(3, 0, 4, 5, 2, 1).reshape(128, BLKF)).astype(NPF16)


def kernel(x, w1, bn1_g, bn1_b, bn1_m, bn1_v, wr, br, w_e,
           bn2_g, bn2_b, bn2_m, bn2_v):
    global LAST_EXEC_NS
    f32 = np.float32
    x = np.ascontiguousarray(np.asarray(x, f32))
    w1 = np.asarray(w1, f32)
    wr = np.asarray(wr, f32)
    br = np.asarray(br, f32)
    w_e = np.asarray(w_e, f32)

    s1 = np.asarray(bn1_g, f32) / np.sqrt(np.asarray(bn1_v, f32) + EPS)
    b1 = np.asarray(bn1_b, f32) - np.asarray(bn1_m, f32) * s1
    s2 = np.asarray(bn2_g, f32) / np.sqrt(np.asarray(bn2_v, f32) + EPS)
    b2 = np.asarray(bn2_b, f32) - np.asarray(bn2_m, f32) * s2

    # conv1 weights (BN1 scale folded) -> vertical-winograd lhsT blocks
    w1sb = _pack_winograd(w1 * s1[:, None, None, None])
    # expert bank likewise (BN2 scale folded); transform commutes with the
    # routing mix, so each expert is pre-transformed on the host
    wef = w_e.reshape(E, C, C, KH, KW) * s2[None, :, None, None, None]
    bank = np.concatenate(
        [_pack_winograd(wef[e]) for e in range(E)], axis=1)
    bank = np.ascontiguousarray(bank)

    # routing weights with the 1/(H*W) mean folded in: [p, (ci, e)]
    wrt = np.ascontiguousarray(
        (wr / HWF).reshape(E, CT, 128).transpose(2, 1, 0).reshape(128, CT * E))
    brb = np.ascontiguousarray(np.broadcast_to(br, (128, E)))
    b1sb = np.ascontiguousarray(b1.reshape(CT, 128).T)
    b2sb = np.ascontiguousarray(b2.reshape(CT, 128).T)

    # padded fp16 x for the conv matmuls (also reused as the residual)
    pad = np.zeros((B, CT, 128, PD, PD), f32)
    pad[:, :, :, 1:H + 1, 1:W + 1] = x.reshape(B, CT, 128, H, W)
    xpad = np.ascontiguousarray(pad.reshape(B, CT, 128, PF).astype(NPF16))

    nc = _get_program()
    in_maps = []
    for c in range(NCORES):
        sl = slice(S * c, S * (c + 1))
        in_maps.append({
            "xpad": np.ascontiguousarray(xpad[sl]),
            "w1sb": w1sb, "bank": bank, "wrt": wrt, "brb": brb,
            "b1sb": b1sb, "b2sb": b2sb,
        })

    res = run_bass_kernel_spmd(
        nc, in_maps, core_ids=list(range(NCORES)), trace=TRACE)
    LAST_EXEC_NS = res.exec_time_ns

    out = np.empty((B, C, H, W), f32)
    for c in range(NCORES):
        out[S * c:S * (c + 1)] = res.results[c]["out"].reshape(S, C, H, W)
    return out


if __name__ == "__main__":
    rng = np.random.default_rng(0)
    ins = {
        "x": rng.standard_normal((B, C, H, W), f32 := np.float32),
        "w1": rng.standard_normal((C, C, KH, KW), f32) * 0.05,
        "bn1_g": np.ones(C, f32), "bn1_b": np.zeros(C, f32),
        "bn1_m": rng.standard_normal(C, f32) * 0.05,
        "bn1_v": np.abs(rng.standard_normal(C, f32) * 0.05) + 1.0,
        "wr": rng.standard_normal((E, C), f32) * 0.05,
        "br": np.zeros(E, f32),
        "w_e": rng.standard_normal((E, C * C * KH * KW), f32) * 0.05,
        "bn2_g": np.ones(C, f32), "bn2_b": np.zeros(C, f32),
        "bn2_m": rng.standard_normal(C, f32) * 0.05,
        "bn2_v": np.abs(rng.standard_normal(C, f32) * 0.05) + 1.0,
    }
    o = kernel(**ins)
    print(o.shape, o.dtype)
